# revision 12
# baseline (speedup 1.0000x reference)
"""Trainium2 Bass kernel for nn_CovaMLoss.

Computes sim[b,k,n] = sum_{c,d} qhat[b,c,n] * S[k,c,d] * qhat[b,d,n] where
qhat is the per-(b,c)-row L2-normalized input reshaped to [B, C, H*W], and
returns sim reshaped to [B, 1, K*H*W].

Strategy (per core, data-parallel over B across 8 cores):
  Host: symmetrize each S_k, eigendecompose, build W[c, (k,i)] = V_k[:,i] *
  sqrt(|lam_ki|) so that sim[k,n] = sum_i sign_ki * (W[:,ki] . qhat[:,n])^2.
  Device: P = W^T qhat via row-tiled (contract=32) PE matmuls into PSUM,
  square via ACT/DVE on the PSUM->SBUF drain, then reduce over i with
  sign-carrying mask matmuls (PSUM accumulation over slot groups).
  Row norms ride on an ACT Square+accum pass over q plus one tiny
  fold/replicate matmul; 1/norm is folded into per-batch scaled W.
"""

import sys

for _p in ("/opt/trn_rl_repo", "/root/.axon_site/_ro/trn_rl_repo"):
    if _p not in sys.path:
        sys.path.append(_p)

from contextlib import ExitStack

import numpy as np

import concourse.bass as bass  # noqa: F401  (bass must import before tile)
import concourse.tile as tile
from concourse import bacc, bass_utils, mybir

B, C, H, W, K = 64, 32, 64, 64, 16
N = H * W                  # 4096
NCORES = 8
BPC = B // NCORES          # 8 batches per core
S = 4                      # n-superblocks stacked on partitions
FPB = N // S               # 1024 free elems per s-block
CHUNK = 512                # matmul moving-operand chunk (one PSUM bank)
KC = K * C                 # 512 slots
G = KC // 128              # 4 slot groups of 128

F32 = mybir.dt.float32
F32R = mybir.dt.float32r
BF16 = mybir.dt.bfloat16
AF = mybir.ActivationFunctionType


def _host_prep(covas: np.ndarray):
    """Eigen-decompose symmetrized covas into sqrt-scaled directions."""
    Wmat = np.zeros((C, KC), np.float64)
    sign = np.zeros(KC, np.float64)
    for k in range(K):
        T = (covas[k].astype(np.float64) + covas[k].astype(np.float64).T) / 2.0
        lam, V = np.linalg.eigh(T)
        Wmat[:, k * C:(k + 1) * C] = V * np.sqrt(np.abs(lam))[None, :]
        sign[k * C:(k + 1) * C] = np.sign(lam)
    # W4[32*s + c, j] = W[c, j], replicated over the 4 s-blocks
    W4 = np.tile(Wmat.astype(np.float32), (S, 1))                  # [128, 512]
    # masks[j_local, 32*g + k] = sign for slot (128*g + j_local) when that
    # slot's k matches; 32 columns per group (16 real k's + 16 zeros so the
    # mask matmul initializes the full 32-partition sim stripe).
    masks = np.zeros((128, 32 * G), np.float32)  # cast to bf16 below
    for g in range(G):
        for j in range(128):
            slot = 128 * g + j
            masks[j, 32 * g + slot // C] = sign[slot]
    # foldrep[32*s + c, 32*s' + c'] = (c == c'): one matmul that both sums
    # the per-s-block partial norms and re-replicates to all 128 partitions.
    foldrep = np.tile(np.eye(C, dtype=np.float32), (S, S))         # [128, 128]
    import ml_dtypes
    return W4, masks.astype(ml_dtypes.bfloat16), foldrep


def _build_kernel():
    nc = bacc.Bacc(
        "TRN2",
        target_bir_lowering=False,
        debug=False,
        enable_asserts=True,
        num_devices=NCORES,
    )
    q_ap = nc.dram_tensor("q", [BPC, C, N], F32R, kind="ExternalInput").ap()
    w4_ap = nc.dram_tensor("w4", [128, KC], F32, kind="ExternalInput").ap()
    mk_ap = nc.dram_tensor("masks", [128, 32 * G], BF16, kind="ExternalInput").ap()
    fr_ap = nc.dram_tensor("foldrep", [128, 128], F32, kind="ExternalInput").ap()
    # Raw stage dumps [b, m, 128, 512]; host unshuffles (k,s,m) -> [b, k, n].
    out_ap = nc.dram_tensor(
        "sim_raw", [BPC, FPB // CHUNK, 128, CHUNK], F32, kind="ExternalOutput"
    ).ap()

    with tile.TileContext(nc) as tc, ExitStack() as ctx:
        const = ctx.enter_context(tc.tile_pool(name="const", bufs=1))
        qpool = ctx.enter_context(tc.tile_pool(name="qpool", bufs=2))
        scr_pool = ctx.enter_context(tc.tile_pool(name="scr", bufs=2))
        nrm_pool = ctx.enter_context(tc.tile_pool(name="nrm", bufs=4))
        wb_pool = ctx.enter_context(tc.tile_pool(name="wb", bufs=2))
        p2_pool = ctx.enter_context(tc.tile_pool(name="p2", bufs=4))
        stage_pool = ctx.enter_context(tc.tile_pool(name="stage", bufs=2))
        tmp_pool = ctx.enter_context(tc.tile_pool(name="tmp", bufs=2))
        psA = ctx.enter_context(tc.tile_pool(name="psA", bufs=2, space="PSUM"))
        psSim = ctx.enter_context(tc.tile_pool(name="psSim", bufs=2, space="PSUM"))
        psNrm = ctx.enter_context(tc.tile_pool(name="psNrm", bufs=1, space="PSUM"))

        w4 = const.tile([128, KC], F32)
        nc.sync.dma_start(w4[:], w4_ap[:])
        masks = const.tile([128, 32 * G], BF16)
        nc.sync.dma_start(masks[:], mk_ap[:])
        foldrep = const.tile([128, 128], F32)
        nc.sync.dma_start(foldrep[:], fr_ap[:])

        # Round-robin the PSUM->SBUF square-drain between ACT and DVE.
        # ACT tile = 997ns, DVE tile = ~2258ns; ratio ~ 11:5 per 16 tiles.
        drain_dve = {1, 4, 7, 10, 13}

        for b in range(BPC):
            q4 = qpool.tile([128, FPB], F32R)
            nc.sync.dma_start(q4[:], q_ap[b].rearrange("c (s f) -> s c f", s=S))

            # ---- row norms -> rnorm4 [128, 1] (1/norm, replicated per s) --
            scr = scr_pool.tile([128, FPB], F32)
            ss4 = nrm_pool.tile([128, 1], F32)
            nc.scalar.activation(scr[:], q4.bitcast(F32)[:], AF.Square, accum_out=ss4[:])
            nrm2 = psNrm.tile([128, 1], F32)
            nc.tensor.matmul(nrm2[:], lhsT=foldrep[:], rhs=ss4[:],
                             start=True, stop=True)
            snrm = nrm_pool.tile([128, 1], F32)
            nc.scalar.activation(snrm[:], nrm2[:], AF.Sqrt)
            rnorm = nrm_pool.tile([128, 1], F32)
            nc.vector.reciprocal(rnorm[:], snrm[:])
            wb = wb_pool.tile([128, KC], F32R)
            nc.vector.tensor_scalar_mul(wb[:], w4[:], rnorm[:])

            # ---- main pipeline ----
            for m in range(FPB // CHUNK):          # 2 chunks per s-block
                sim_ps = psSim.tile([128, CHUNK], F32)
                di = 0
                for g in range(G):
                    for half in range(2):          # s-pairs (0,1), (2,3)
                        a_ps = psA.tile([128, 2 * CHUNK], F32)   # 2 banks
                        for si in range(2):
                            s = 2 * half + si
                            nc.tensor.matmul(
                                a_ps[:, si * CHUNK:(si + 1) * CHUNK],
                                lhsT=wb[32 * s:32 * (s + 1),
                                        128 * g:128 * (g + 1)],
                                rhs=q4[32 * s:32 * (s + 1),
                                       m * CHUNK:(m + 1) * CHUNK],
                                start=True, stop=True,
                                tile_position=(32 * s, 0),
                            )
                        p2 = p2_pool.tile([128, 2 * CHUNK], BF16)
                        if di in drain_dve:
                            # DVE can't read two PSUM operands: copy out first.
                            tmp = tmp_pool.tile([128, 2 * CHUNK], F32)
                            nc.vector.tensor_copy(tmp[:], a_ps[:])
                            nc.vector.tensor_mul(p2[:], tmp[:], tmp[:])
                        else:
                            nc.scalar.activation(p2[:], a_ps[:], AF.Square)
                        di += 1
                        for si in range(2):
                            s = 2 * half + si
                            nc.tensor.matmul(
                                sim_ps[32 * s:32 * (s + 1), :],
                                lhsT=masks[:, 32 * g:32 * (g + 1)],
                                rhs=p2[:, si * CHUNK:(si + 1) * CHUNK],
                                start=(g == 0), stop=(g == G - 1),
                                tile_position=(0, 32 * s),
                                skip_group_check=True,
                            )
                stage = stage_pool.tile([128, CHUNK], F32)
                nc.scalar.activation(stage[:], sim_ps[:], AF.Copy)
                # raw[b, m, 32*s + k, f] = sim[b, k, 1024*s + 512*m + f]
                nc.sync.dma_start(out_ap[b, m], stage[:])
    nc.compile()
    return nc


_CACHE = {}


def _get_nc():
    if "nc" not in _CACHE:
        _CACHE["nc"] = _build_kernel()
    return _CACHE["nc"]


def make_in_maps(input_np: np.ndarray, covas_np: np.ndarray):
    q = np.ascontiguousarray(
        np.asarray(input_np, dtype=np.float32).reshape(B, C, N))
    W4, masks, foldrep = _host_prep(np.asarray(covas_np, dtype=np.float32))
    in_maps = []
    for c in range(NCORES):
        in_maps.append({
            "q": np.ascontiguousarray(q[c * BPC:(c + 1) * BPC]),
            "w4": W4,
            "masks": masks,
            "foldrep": foldrep,
        })
    return in_maps


def assemble(results) -> np.ndarray:
    out = np.empty((B, K, N), np.float32)
    for c in range(NCORES):
        raw = results[c]["sim_raw"]                 # [BPC, 2, 128, 512]
        # raw[b, m, 32*s + k, f] -> sim[b, k, 1024*s + 512*m + f]
        r = raw.reshape(BPC, FPB // CHUNK, S, 32, CHUNK)[:, :, :, :K, :]
        out[c * BPC:(c + 1) * BPC] = (
            r.transpose(0, 3, 2, 1, 4).reshape(BPC, K, N))
    return np.ascontiguousarray(out.reshape(B, 1, K * N))


def kernel(input: np.ndarray, support_covas: np.ndarray) -> np.ndarray:
    nc = _get_nc()
    in_maps = make_in_maps(input, support_covas)
    res = bass_utils.run_bass_kernel_spmd(nc, in_maps, core_ids=list(range(NCORES)))
    return assemble(res.results)


if __name__ == "__main__":
    rng = np.random.default_rng(0)
    inp = rng.standard_normal((B, C, H, W)).astype(np.float32)
    cov = rng.standard_normal((K, C, C)).astype(np.float32)
    out = kernel(inp, cov)
    print("kernel output shape:", out.shape, out.dtype)


# revision 15
# speedup vs baseline: 5.4772x; 5.4772x over previous
"""Trainium2 Bass kernel for nn_CovaMLoss.

Computes sim[b,k,n] = sum_{c,d} qhat[b,c,n] * S[k,c,d] * qhat[b,d,n] where
qhat is the per-(b,c)-row L2-normalized input reshaped to [B, C, H*W], and
returns sim reshaped to [B, 1, K*H*W].

Strategy (per core, data-parallel over B across 8 cores):
  Host: symmetrize each S_k, eigendecompose, build W[c, (k,i)] = V_k[:,i] *
  sqrt(|lam_ki|) so that sim[k,n] = sum_i sign_ki * (W[:,ki] . qhat[:,n])^2.
  Device: P = W^T qhat via row-tiled (contract=32) PE matmuls into PSUM,
  square via ACT/DVE on the PSUM->SBUF drain, then reduce over i with
  sign-carrying mask matmuls (PSUM accumulation over slot groups).
  Row norms ride on an ACT Square+accum pass over q plus one tiny
  fold/replicate matmul; 1/norm is folded into per-batch scaled W.
"""

import sys

for _p in ("/opt/trn_rl_repo", "/root/.axon_site/_ro/trn_rl_repo"):
    if _p not in sys.path:
        sys.path.append(_p)

from contextlib import ExitStack

import numpy as np

import concourse.bass as bass  # noqa: F401  (bass must import before tile)
import concourse.tile as tile
from concourse import bacc, bass_utils, mybir

B, C, H, W, K = 64, 32, 64, 64, 16
N = H * W                  # 4096
NCORES = 8
BPC = B // NCORES          # 8 batches per core
S = 4                      # n-superblocks stacked on partitions
FPB = N // S               # 1024 free elems per s-block
CHUNK = 512                # matmul moving-operand chunk (one PSUM bank)
KC = K * C                 # 512 slots
G = KC // 128              # 4 slot groups of 128

F32 = mybir.dt.float32
F32R = mybir.dt.float32r
BF16 = mybir.dt.bfloat16
AF = mybir.ActivationFunctionType


def _host_prep(covas: np.ndarray):
    """Eigen-decompose symmetrized covas into sqrt-scaled directions."""
    Wmat = np.zeros((C, KC), np.float64)
    sign = np.zeros(KC, np.float64)
    for k in range(K):
        T = (covas[k].astype(np.float64) + covas[k].astype(np.float64).T) / 2.0
        lam, V = np.linalg.eigh(T)
        Wmat[:, k * C:(k + 1) * C] = V * np.sqrt(np.abs(lam))[None, :]
        sign[k * C:(k + 1) * C] = np.sign(lam)
    # W4[32*s + c, j] = W[c, j], replicated over the 4 s-blocks
    W4 = np.tile(Wmat.astype(np.float32), (S, 1))                  # [128, 512]
    # masks[j_local, 32*g + k] = sign for slot (128*g + j_local) when that
    # slot's k matches; 32 columns per group (16 real k's + 16 zeros so the
    # mask matmul initializes the full 32-partition sim stripe).
    masks = np.zeros((128, 32 * G), np.float32)  # cast to bf16 below
    for g in range(G):
        for j in range(128):
            slot = 128 * g + j
            masks[j, 32 * g + slot // C] = sign[slot]
    # foldrep[32*s + c, 32*s' + c'] = (c == c'): one matmul that both sums
    # the per-s-block partial norms and re-replicates to all 128 partitions.
    foldrep = np.tile(np.eye(C, dtype=np.float32), (S, S))         # [128, 128]
    import ml_dtypes
    return W4, masks.astype(ml_dtypes.bfloat16), foldrep


def _build_kernel(repeat: int = 1, drain_dve_set=None):
    nc = bacc.Bacc(
        "TRN2",
        target_bir_lowering=False,
        debug=False,
        enable_asserts=True,
        num_devices=NCORES,
    )
    q_ap = nc.dram_tensor("q", [BPC, C, N], F32R, kind="ExternalInput").ap()
    w4_ap = nc.dram_tensor("w4", [128, KC], F32, kind="ExternalInput").ap()
    mk_ap = nc.dram_tensor("masks", [128, 32 * G], BF16, kind="ExternalInput").ap()
    fr_ap = nc.dram_tensor("foldrep", [128, 128], F32, kind="ExternalInput").ap()
    # Raw stage dumps [b, m, 128, 512]; host unshuffles (k,s,m) -> [b, k, n].
    out_ap = nc.dram_tensor(
        "sim_raw", [BPC, FPB // CHUNK, 128, CHUNK], F32, kind="ExternalOutput"
    ).ap()

    with tile.TileContext(nc) as tc, ExitStack() as ctx:
        const = ctx.enter_context(tc.tile_pool(name="const", bufs=1))
        qpool = ctx.enter_context(tc.tile_pool(name="qpool", bufs=2))
        scr_pool = ctx.enter_context(tc.tile_pool(name="scr", bufs=2))
        nrm_pool = ctx.enter_context(tc.tile_pool(name="nrm", bufs=4))
        wb_pool = ctx.enter_context(tc.tile_pool(name="wb", bufs=2))
        p2_pool = ctx.enter_context(tc.tile_pool(name="p2", bufs=4))
        stage_pool = ctx.enter_context(tc.tile_pool(name="stage", bufs=2))
        tmp_pool = ctx.enter_context(tc.tile_pool(name="tmp", bufs=2))
        psA = ctx.enter_context(tc.tile_pool(name="psA", bufs=2, space="PSUM"))
        psSim = ctx.enter_context(tc.tile_pool(name="psSim", bufs=2, space="PSUM"))
        psNrm = ctx.enter_context(tc.tile_pool(name="psNrm", bufs=1, space="PSUM"))

        w4 = const.tile([128, KC], F32)
        nc.sync.dma_start(w4[:], w4_ap[:])
        masks = const.tile([128, 32 * G], BF16)
        nc.sync.dma_start(masks[:], mk_ap[:])
        foldrep = const.tile([128, 128], F32)
        nc.sync.dma_start(foldrep[:], fr_ap[:])

        # Round-robin the PSUM->SBUF square-drain between ACT and DVE.
        # ACT tile = 997ns, DVE tile = ~2258ns; ratio ~ 11:5 per 16 tiles.
        # Empirical: keeping the whole PSUM->SBUF square-drain on ACT beats
        # an ACT/DVE split (DVE needs a copy+mul pair per tile and its DRAINs
        # lengthen the drain->mask-matmul chain).
        drain_dve = set() if drain_dve_set is None else drain_dve_set

        for b_iter in range(BPC * repeat):
            b = b_iter % BPC
            q4 = qpool.tile([128, FPB], F32R)
            nc.sync.dma_start(q4[:], q_ap[b].rearrange("c (s f) -> s c f", s=S))

            # ---- row norms -> rnorm4 [128, 1] (1/norm, replicated per s) --
            scr = scr_pool.tile([128, FPB], F32)
            ss4 = nrm_pool.tile([128, 1], F32)
            nc.scalar.activation(scr[:], q4.bitcast(F32)[:], AF.Square, accum_out=ss4[:])
            nrm2 = psNrm.tile([128, 1], F32)
            nc.tensor.matmul(nrm2[:], lhsT=foldrep[:], rhs=ss4[:],
                             start=True, stop=True)
            snrm = nrm_pool.tile([128, 1], F32)
            nc.scalar.activation(snrm[:], nrm2[:], AF.Sqrt)
            rnorm = nrm_pool.tile([128, 1], F32)
            nc.vector.reciprocal(rnorm[:], snrm[:])
            wb = wb_pool.tile([128, KC], F32R)
            nc.vector.tensor_scalar_mul(wb[:], w4[:], rnorm[:])

            # ---- main pipeline ----
            for m in range(FPB // CHUNK):          # 2 chunks per s-block
                sim_ps = psSim.tile([128, CHUNK], F32)
                di = 0
                for g in range(G):
                    for half in range(2):          # s-pairs (0,1), (2,3)
                        a_ps = psA.tile([128, 2 * CHUNK], F32)   # 2 banks
                        for si in range(2):
                            s = 2 * half + si
                            nc.tensor.matmul(
                                a_ps[:, si * CHUNK:(si + 1) * CHUNK],
                                lhsT=wb[32 * s:32 * (s + 1),
                                        128 * g:128 * (g + 1)],
                                rhs=q4[32 * s:32 * (s + 1),
                                       m * CHUNK:(m + 1) * CHUNK],
                                start=True, stop=True,
                                tile_position=(32 * s, 0),
                            )
                        p2 = p2_pool.tile([128, 2 * CHUNK], BF16)
                        if di in drain_dve:
                            # DVE can't read two PSUM operands: copy out first.
                            tmp = tmp_pool.tile([128, 2 * CHUNK], F32)
                            nc.vector.tensor_copy(tmp[:], a_ps[:])
                            nc.vector.tensor_mul(p2[:], tmp[:], tmp[:])
                        else:
                            nc.scalar.activation(p2[:], a_ps[:], AF.Square)
                        di += 1
                        for si in range(2):
                            s = 2 * half + si
                            nc.tensor.matmul(
                                sim_ps[32 * s:32 * (s + 1), :],
                                lhsT=masks[:, 32 * g:32 * (g + 1)],
                                rhs=p2[:, si * CHUNK:(si + 1) * CHUNK],
                                start=(g == 0), stop=(g == G - 1),
                                tile_position=(0, 32 * s),
                                skip_group_check=True,
                            )
                stage = stage_pool.tile([128, CHUNK], F32)
                nc.scalar.activation(stage[:], sim_ps[:], AF.Copy)
                # raw[b, m, 32*s + k, f] = sim[b, k, 1024*s + 512*m + f]
                nc.sync.dma_start(out_ap[b, m], stage[:])
    nc.compile()
    return nc


_CACHE = {}


def _get_nc(repeat: int = 1, drain_dve_set=None):
    key = ("nc", repeat, None if drain_dve_set is None else tuple(sorted(drain_dve_set)))
    if key not in _CACHE:
        _CACHE[key] = _build_kernel(repeat, drain_dve_set)
    return _CACHE[key]


def make_in_maps(input_np: np.ndarray, covas_np: np.ndarray):
    q = np.ascontiguousarray(
        np.asarray(input_np, dtype=np.float32).reshape(B, C, N))
    W4, masks, foldrep = _host_prep(np.asarray(covas_np, dtype=np.float32))
    in_maps = []
    for c in range(NCORES):
        in_maps.append({
            "q": np.ascontiguousarray(q[c * BPC:(c + 1) * BPC]),
            "w4": W4,
            "masks": masks,
            "foldrep": foldrep,
        })
    return in_maps


def assemble(results) -> np.ndarray:
    out = np.empty((B, K, N), np.float32)
    for c in range(NCORES):
        raw = results[c]["sim_raw"]                 # [BPC, 2, 128, 512]
        # raw[b, m, 32*s + k, f] -> sim[b, k, 1024*s + 512*m + f]
        r = raw.reshape(BPC, FPB // CHUNK, S, 32, CHUNK)[:, :, :, :K, :]
        out[c * BPC:(c + 1) * BPC] = (
            r.transpose(0, 3, 2, 1, 4).reshape(BPC, K, N))
    return np.ascontiguousarray(out.reshape(B, 1, K * N))


def kernel(input: np.ndarray, support_covas: np.ndarray) -> np.ndarray:
    nc = _get_nc()
    in_maps = make_in_maps(input, support_covas)
    res = bass_utils.run_bass_kernel_spmd(nc, in_maps, core_ids=list(range(NCORES)))
    return assemble(res.results)


if __name__ == "__main__":
    rng = np.random.default_rng(0)
    inp = rng.standard_normal((B, C, H, W)).astype(np.float32)
    cov = rng.standard_normal((K, C, C)).astype(np.float32)
    out = kernel(inp, cov)
    print("kernel output shape:", out.shape, out.dtype)


# revision 17
# speedup vs baseline: 11.4639x; 2.0930x over previous
"""Trainium2 Bass kernel for nn_CovaMLoss.

Computes sim[b,k,n] = sum_{c,d} qhat[b,c,n] * S[k,c,d] * qhat[b,d,n] where
qhat is the per-(b,c)-row L2-normalized input reshaped to [B, C, H*W], and
returns sim reshaped to [B, 1, K*H*W].

Strategy (per core, data-parallel over B across 8 cores):
  Host: symmetrize each S_k, eigendecompose, build W[c, (k,i)] = V_k[:,i] *
  sqrt(|lam_ki|) so that sim[k,n] = sum_i sign_ki * (W[:,ki] . qhat[:,n])^2.
  Device: P = W^T qhat via row-tiled (contract=32) PE matmuls into PSUM,
  square via ACT/DVE on the PSUM->SBUF drain, then reduce over i with
  sign-carrying mask matmuls (PSUM accumulation over slot groups).
  Row norms ride on an ACT Square+accum pass over q plus one tiny
  fold/replicate matmul; 1/norm is folded into per-batch scaled W.
"""

import sys

for _p in ("/opt/trn_rl_repo", "/root/.axon_site/_ro/trn_rl_repo"):
    if _p not in sys.path:
        sys.path.append(_p)

from contextlib import ExitStack

import numpy as np

import concourse.bass as bass  # noqa: F401  (bass must import before tile)
import concourse.tile as tile
from concourse import bacc, bass_utils, mybir

B, C, H, W, K = 64, 32, 64, 64, 16
N = H * W                  # 4096
NCORES = 8
BPC = B // NCORES          # 8 batches per core
S = 4                      # n-superblocks stacked on partitions
FPB = N // S               # 1024 free elems per s-block
CHUNK = 512                # matmul moving-operand chunk (one PSUM bank)
KC = K * C                 # 512 slots
G = KC // 128              # 4 slot groups of 128

F32 = mybir.dt.float32
F32R = mybir.dt.float32r
BF16 = mybir.dt.bfloat16
AF = mybir.ActivationFunctionType


def _host_prep(covas: np.ndarray):
    """Eigen-decompose symmetrized covas into sqrt-scaled directions."""
    Wmat = np.zeros((C, KC), np.float64)
    sign = np.zeros(KC, np.float64)
    for k in range(K):
        T = (covas[k].astype(np.float64) + covas[k].astype(np.float64).T) / 2.0
        lam, V = np.linalg.eigh(T)
        Wmat[:, k * C:(k + 1) * C] = V * np.sqrt(np.abs(lam))[None, :]
        sign[k * C:(k + 1) * C] = np.sign(lam)
    # W4[32*s + c, j] = W[c, j], replicated over the 4 s-blocks
    W4 = np.tile(Wmat.astype(np.float32), (S, 1))                  # [128, 512]
    # masks[j_local, 32*g + k] = sign for slot (128*g + j_local) when that
    # slot's k matches; 32 columns per group (16 real k's + 16 zeros so the
    # mask matmul initializes the full 32-partition sim stripe).
    masks = np.zeros((128, 32 * G), np.float32)  # cast to bf16 below
    for g in range(G):
        for j in range(128):
            slot = 128 * g + j
            masks[j, 32 * g + slot // C] = sign[slot]
    # foldrep[32*s + c, 32*s' + c'] = (c == c'): one matmul that both sums
    # the per-s-block partial norms and re-replicates to all 128 partitions.
    foldrep = np.tile(np.eye(C, dtype=np.float32), (S, S))         # [128, 128]
    import ml_dtypes
    return W4, masks.astype(ml_dtypes.bfloat16), foldrep


def _build_kernel(repeat: int = 1, drain_dve_set=None):
    nc = bacc.Bacc(
        "TRN2",
        target_bir_lowering=False,
        debug=False,
        enable_asserts=True,
        num_devices=NCORES,
    )
    q_ap = nc.dram_tensor("q", [BPC, C, N], F32R, kind="ExternalInput").ap()
    w4_ap = nc.dram_tensor("w4", [128, KC], F32, kind="ExternalInput").ap()
    mk_ap = nc.dram_tensor("masks", [128, 32 * G], BF16, kind="ExternalInput").ap()
    fr_ap = nc.dram_tensor("foldrep", [128, 128], F32, kind="ExternalInput").ap()
    # Raw stage dumps [b, m, 128, 512]; host unshuffles (k,s,m) -> [b, k, n].
    out_ap = nc.dram_tensor(
        "sim_raw", [BPC, FPB // CHUNK, 128, CHUNK], F32, kind="ExternalOutput"
    ).ap()

    with tile.TileContext(nc) as tc, ExitStack() as ctx:
        const = ctx.enter_context(tc.tile_pool(name="const", bufs=1))
        qpool = ctx.enter_context(tc.tile_pool(name="qpool", bufs=2))
        scr_pool = ctx.enter_context(tc.tile_pool(name="scr", bufs=2))
        nrm_pool = ctx.enter_context(tc.tile_pool(name="nrm", bufs=4))
        wb_pool = ctx.enter_context(tc.tile_pool(name="wb", bufs=2))
        p2_pool = ctx.enter_context(tc.tile_pool(name="p2", bufs=4))
        stage_pool = ctx.enter_context(tc.tile_pool(name="stage", bufs=2))
        tmp_pool = ctx.enter_context(tc.tile_pool(name="tmp", bufs=2))
        psA = ctx.enter_context(tc.tile_pool(name="psA", bufs=2, space="PSUM"))
        psSim = ctx.enter_context(tc.tile_pool(name="psSim", bufs=2, space="PSUM"))
        psNrm = ctx.enter_context(tc.tile_pool(name="psNrm", bufs=1, space="PSUM"))

        w4 = const.tile([128, KC], F32)
        nc.sync.dma_start(w4[:], w4_ap[:])
        masks = const.tile([128, 32 * G], BF16)
        nc.sync.dma_start(masks[:], mk_ap[:])
        foldrep = const.tile([128, 128], F32)
        nc.sync.dma_start(foldrep[:], fr_ap[:])

        # Round-robin the PSUM->SBUF square-drain between ACT and DVE.
        # ACT tile = 997ns, DVE tile = ~2258ns; ratio ~ 11:5 per 16 tiles.
        # Empirical: keeping the whole PSUM->SBUF square-drain on ACT beats
        # an ACT/DVE split (DVE needs a copy+mul pair per tile and its DRAINs
        # lengthen the drain->mask-matmul chain).
        drain_dve = set() if drain_dve_set is None else drain_dve_set

        for b_iter in range(BPC * repeat):
            b = b_iter % BPC
            q4 = qpool.tile([128, FPB], F32R)
            nc.sync.dma_start(q4[:], q_ap[b].rearrange("c (s f) -> s c f", s=S))

            # ---- row norms -> rnorm4 [128, 1] (1/norm, replicated per s) --
            scr = scr_pool.tile([128, FPB], F32)
            ss4 = nrm_pool.tile([128, 1], F32)
            nc.scalar.activation(scr[:], q4.bitcast(F32)[:], AF.Square, accum_out=ss4[:])
            nrm2 = psNrm.tile([128, 1], F32)
            nc.tensor.matmul(nrm2[:], lhsT=foldrep[:], rhs=ss4[:],
                             start=True, stop=True)
            snrm = nrm_pool.tile([128, 1], F32)
            nc.scalar.activation(snrm[:], nrm2[:], AF.Sqrt)
            rnorm = nrm_pool.tile([128, 1], F32)
            nc.vector.reciprocal(rnorm[:], snrm[:])
            wb = wb_pool.tile([128, KC], F32R)
            nc.vector.tensor_scalar_mul(wb[:], w4[:], rnorm[:])

            # ---- main pipeline ----
            for m in range(FPB // CHUNK):          # 2 chunks per s-block
                sim_ps = psSim.tile([128, CHUNK], F32)
                di = 0
                for g in range(G):
                    for half in range(2):          # s-pairs (0,1), (2,3)
                        a_ps = psA.tile([128, 2 * CHUNK], F32)   # 2 banks
                        for si in range(2):
                            s = 2 * half + si
                            nc.tensor.matmul(
                                a_ps[:, si * CHUNK:(si + 1) * CHUNK],
                                lhsT=wb[32 * s:32 * (s + 1),
                                        128 * g:128 * (g + 1)],
                                rhs=q4[32 * s:32 * (s + 1),
                                       m * CHUNK:(m + 1) * CHUNK],
                                start=True, stop=True,
                                tile_position=(32 * s, 0),
                            )
                        p2 = p2_pool.tile([128, 2 * CHUNK], BF16)
                        if di in drain_dve:
                            # DVE can't read two PSUM operands: copy out first.
                            tmp = tmp_pool.tile([128, 2 * CHUNK], F32)
                            nc.vector.tensor_copy(tmp[:], a_ps[:])
                            nc.vector.tensor_mul(p2[:], tmp[:], tmp[:])
                        else:
                            nc.scalar.activation(p2[:], a_ps[:], AF.Square)
                        di += 1
                        for si in range(2):
                            s = 2 * half + si
                            nc.tensor.matmul(
                                sim_ps[32 * s:32 * (s + 1), :],
                                lhsT=masks[:, 32 * g:32 * (g + 1)],
                                rhs=p2[:, si * CHUNK:(si + 1) * CHUNK],
                                start=(g == 0), stop=(g == G - 1),
                                tile_position=(0, 32 * s),
                                skip_group_check=True,
                            )
                stage = stage_pool.tile([128, CHUNK], F32)
                nc.vector.tensor_copy(stage[:], sim_ps[:])
                # raw[b, m, 32*s + k, f] = sim[b, k, 1024*s + 512*m + f]
                nc.sync.dma_start(out_ap[b, m], stage[:])
    nc.compile()
    return nc


_CACHE = {}


def _get_nc(repeat: int = 1, drain_dve_set=None):
    key = ("nc", repeat, None if drain_dve_set is None else tuple(sorted(drain_dve_set)))
    if key not in _CACHE:
        _CACHE[key] = _build_kernel(repeat, drain_dve_set)
    return _CACHE[key]


def make_in_maps(input_np: np.ndarray, covas_np: np.ndarray):
    q = np.ascontiguousarray(
        np.asarray(input_np, dtype=np.float32).reshape(B, C, N))
    W4, masks, foldrep = _host_prep(np.asarray(covas_np, dtype=np.float32))
    in_maps = []
    for c in range(NCORES):
        in_maps.append({
            "q": np.ascontiguousarray(q[c * BPC:(c + 1) * BPC]),
            "w4": W4,
            "masks": masks,
            "foldrep": foldrep,
        })
    return in_maps


def assemble(results) -> np.ndarray:
    out = np.empty((B, K, N), np.float32)
    for c in range(NCORES):
        raw = results[c]["sim_raw"]                 # [BPC, 2, 128, 512]
        # raw[b, m, 32*s + k, f] -> sim[b, k, 1024*s + 512*m + f]
        r = raw.reshape(BPC, FPB // CHUNK, S, 32, CHUNK)[:, :, :, :K, :]
        out[c * BPC:(c + 1) * BPC] = (
            r.transpose(0, 3, 2, 1, 4).reshape(BPC, K, N))
    return np.ascontiguousarray(out.reshape(B, 1, K * N))


def kernel(input: np.ndarray, support_covas: np.ndarray) -> np.ndarray:
    nc = _get_nc()
    in_maps = make_in_maps(input, support_covas)
    res = bass_utils.run_bass_kernel_spmd(nc, in_maps, core_ids=list(range(NCORES)))
    return assemble(res.results)


if __name__ == "__main__":
    rng = np.random.default_rng(0)
    inp = rng.standard_normal((B, C, H, W)).astype(np.float32)
    cov = rng.standard_normal((K, C, C)).astype(np.float32)
    out = kernel(inp, cov)
    print("kernel output shape:", out.shape, out.dtype)
